# revision 5
# baseline (speedup 1.0000x reference)
"""Trainium2 Bass kernel for nn_EventSampler (Hawkes thinning sampler).

Math (per (b,l) row, fully independent):
  bound = 1.5 * sum_m softplus(mu_m + alpha_m * gamma[type] * exp(-beta_m * t))
          maximized over t in linspace(0,5,10).  Since alpha,gamma,beta > 0 the
          max is at t=0, so bound = 1.5 * sum_m softplus(mu_m + alpha_m*gamma[type]).
  exp_j = cumsum(-log1p(-e_unif) / bound)               [E]
  intens_total[e] = sum_m softplus(mu_m + alpha_m*g*exp(-beta_m*exp_j[e]))
  accept[k,e] = u[k,e]*bound / intens_total[e] < 1
  res[k] = exp_j[first accepted e]  (0 if none), clamped to 1e5.

Key reformulation: exp_j is non-decreasing along e, so the first accepted
exp_j equals min over accepted e of exp_j[e].  That turns the
argmax+gather into a masked min-reduction:
  val[k,e] = reject[k,e]*BIG + exp_j[e];  res[k] = min_e val[k,e]
with "none accepted" detectable as res >= BIG.

Sharding: data-parallel over the 8192 (b,l) rows, 1024 rows per core.
"""

import sys
import functools

sys.path.insert(0, "/opt/trn_rl_repo")

import numpy as np

import concourse.bacc as bacc
import concourse.mybir as mybir
import concourse.tile as tile
from concourse.bass_utils import run_bass_kernel_spmd

B, L, E, K, M, NTYPES = 4, 2048, 100, 100, 10, 10
OVER_SAMPLE_RATE = 1.5

NCORES = 8
ROWS = B * L            # 8192 independent (b,l) rows
RPC = ROWS // NCORES    # 1024 rows per core
PT = 128                # rows per partition-tile
NT = RPC // PT          # 8 row-tiles per core
KC = 25                 # k-chunk size for streaming u
NKC = K // KC
BIGF = 1.0e9            # accept/reject decode threshold (> CLAMPF, < any reject val)
CLAMPF = 1.0e8          # exp_j clamp (reference clamps output at 1e5)
HUGE = 2.0 ** 80        # exact power-of-2 scale: rejects land >= ~6e17

F32 = mybir.dt.float32
ALU = mybir.AluOpType
ACTF = mybir.ActivationFunctionType
AX = mybir.AxisListType


def _build(reps: int = 1, gp_n: int = 5):
    """Build the per-core Bass program.

    reps: repeat the whole compute pipeline (for timing via wall-clock slope).
    gp_n: of every 32 u-chunks, how many run their val-pass on GPSIMD
          (load-balancing DVE vs GPSIMD).
    """
    nc = bacc.Bacc()

    eu = nc.dram_tensor("eu", [RPC, E], F32, kind="ExternalInput")
    uu = nc.dram_tensor("uu", [RPC, K, E], F32, kind="ExternalInput")
    tq = nc.dram_tensor("tq", [RPC], F32, kind="ExternalInput")
    mu = nc.dram_tensor("mu", [M], F32, kind="ExternalInput")
    al = nc.dram_tensor("al", [M], F32, kind="ExternalInput")
    be = nc.dram_tensor("be", [M], F32, kind="ExternalInput")
    ga = nc.dram_tensor("ga", [NTYPES], F32, kind="ExternalInput")
    ar = nc.dram_tensor("ar", [NTYPES], F32, kind="ExternalInput")
    res = nc.dram_tensor("res", [RPC, K], F32, kind="ExternalOutput")

    with tile.TileContext(nc) as tc:
        with (
            tc.tile_pool(name="const", bufs=1) as pc,
            tc.tile_pool(name="row", bufs=2) as pr,
            tc.tile_pool(name="uchunk", bufs=3) as pu,
            tc.tile_pool(name="mask", bufs=3) as pm,
            tc.tile_pool(name="val", bufs=3) as pv,
        ):
            # ---- phase 0: per-row constants ----------------------------------
            tga = pc.tile([PT, NTYPES], F32)
            tmu = pc.tile([PT, M], F32)
            tal = pc.tile([PT, M], F32)
            tbe = pc.tile([PT, M], F32)
            tar = pc.tile([PT, NTYPES], F32)
            ttq = pc.tile([PT, NT], F32)
            nc.sync.dma_start(tga[:], ga[:].unsqueeze(0).broadcast_to([PT, NTYPES]))
            nc.sync.dma_start(tmu[:], mu[:].unsqueeze(0).broadcast_to([PT, M]))
            nc.sync.dma_start(tal[:], al[:].unsqueeze(0).broadcast_to([PT, M]))
            nc.sync.dma_start(tbe[:], be[:].unsqueeze(0).broadcast_to([PT, M]))
            nc.sync.dma_start(tar[:], ar[:].unsqueeze(0).broadcast_to([PT, NTYPES]))
            nc.sync.dma_start(ttq[:], tq[:].rearrange("(t p) -> p t", p=PT))

            tnb = pc.tile([PT, M], F32)
            nc.vector.tensor_scalar_mul(tnb[:], tbe[:], -1.0)

            g_all = pc.tile([PT, NT], F32)
            ag_all = pc.tile([PT, NT, M], F32)
            bound_all = pc.tile([PT, NT], F32)
            nrb_all = pc.tile([PT, NT], F32)
            for t in range(NT):
                toh = pr.tile([PT, NTYPES], F32, tag="toh")
                nc.vector.tensor_scalar(
                    toh[:], tar[:], ttq[:, t : t + 1], None, op0=ALU.is_equal
                )
                tgm = pr.tile([PT, NTYPES], F32, tag="tgm")
                nc.vector.tensor_tensor(tgm[:], toh[:], tga[:], op=ALU.mult)
                nc.vector.tensor_reduce(
                    g_all[:, t : t + 1], tgm[:], axis=AX.X, op=ALU.add
                )
                nc.vector.tensor_scalar_mul(
                    ag_all[:, t, :], tal[:], g_all[:, t : t + 1]
                )
                # bound = 1.5 * sum_m softplus(mu + alpha*g)  (max over t at t=0)
                tin = pr.tile([PT, M], F32, tag="tin")
                nc.vector.tensor_tensor(tin[:], ag_all[:, t, :], tmu[:], op=ALU.add)
                te3 = pr.tile([PT, M], F32, tag="te3")
                nc.scalar.activation(te3[:], tin[:], ACTF.Exp)
                tsp = pr.tile([PT, M], F32, tag="tsp")
                nc.scalar.activation(tsp[:], te3[:], ACTF.Ln, bias=1.0)
                tbs = pr.tile([PT, 1], F32, tag="tbs")
                nc.vector.tensor_reduce(tbs[:], tsp[:], axis=AX.X, op=ALU.add)
                nc.vector.tensor_scalar_mul(
                    bound_all[:, t : t + 1], tbs[:], OVER_SAMPLE_RATE
                )
            trb = pc.tile([PT, NT], F32)
            nc.vector.reciprocal(trb[:], bound_all[:])
            nc.vector.tensor_scalar_mul(nrb_all[:], trb[:], -1.0)
            boundH_all = pc.tile([PT, NT], F32)
            nc.vector.tensor_scalar_mul(boundH_all[:], bound_all[:], HUGE)

            # ---- per row-tile pipeline --------------------------------------
            chunk_idx = 0
            for rep in range(reps):
                for t in range(NT):
                    sl = slice(t * PT, (t + 1) * PT)
                    # phase 1: exp_j and intens_total for these 128 rows
                    teu = pr.tile([PT, E], F32, tag="teu")
                    nc.sync.dma_start(teu[:], eu[sl, :])
                    tlg = pr.tile([PT, E], F32, tag="tlg")
                    nc.scalar.activation(tlg[:], teu[:], ACTF.Ln, bias=1.0, scale=-1.0)
                    tjp = pr.tile([PT, E], F32, tag="tjp")
                    nc.vector.tensor_scalar_mul(tjp[:], tlg[:], nrb_all[:, t : t + 1])
                    tex = pr.tile([PT, E], F32, tag="tex")
                    nc.vector.tensor_tensor_scan(
                        tex[:], tjp[:], tjp[:], 0.0, op0=ALU.add, op1=ALU.bypass
                    )
                    texc = pr.tile([PT, E], F32, tag="texc")
                    nc.vector.tensor_scalar_min(texc[:], tex[:], CLAMPF)

                    spm = pr.tile([PT, E, M], F32, tag="spm")
                    for m in range(M):
                        te2 = pr.tile([PT, E], F32, tag="te2")
                        nc.scalar.activation(
                            te2[:], texc[:], ACTF.Exp, scale=tnb[:, m : m + 1]
                        )
                        te4 = pr.tile([PT, E], F32, tag="te4")
                        nc.scalar.activation(
                            te4[:],
                            te2[:],
                            ACTF.Exp,
                            scale=ag_all[:, t, m : m + 1],
                            bias=tmu[:, m : m + 1],
                        )
                        nc.scalar.activation(spm[:, :, m], te4[:], ACTF.Ln, bias=1.0)
                    tint = pr.tile([PT, E], F32, tag="tint")
                    nc.vector.tensor_reduce(tint[:], spm[:], axis=AX.X, op=ALU.add)
                    tintH = pr.tile([PT, E], F32, tag="tintH")
                    nc.vector.tensor_scalar_mul(tintH[:], tint[:], HUGE)

                    # phase 2: stream u, signed reject margin, masked min of exp_j
                    tred = pr.tile([PT, K], F32, tag="tred")
                    tintH_bc = tintH[:].unsqueeze(1).broadcast_to([PT, KC, E])
                    texc_bc = texc[:].unsqueeze(1).broadcast_to([PT, KC, E])
                    for c in range(NKC):
                        tu = pu.tile([PT, KC, E], F32)
                        nc.sync.dma_start(tu[:], uu[sl, c * KC : (c + 1) * KC, :])
                        tacc = pm.tile([PT, KC, E], F32)
                        # d = u*bound*2^80 - intens*2^80  (<0 accept, >=0 reject)
                        nc.vector.scalar_tensor_tensor(
                            tacc[:],
                            tu[:],
                            boundH_all[:, t : t + 1],
                            tintH_bc,
                            op0=ALU.mult,
                            op1=ALU.subtract,
                        )
                        tval = pv.tile([PT, KC, E], F32)
                        # accept -> exp_j ; reject -> d (>= ~6e17, decodes as none)
                        # (gpsimd offload impossible: walrus rejects TensorTensor on Pool)
                        nc.vector.tensor_tensor(tval[:], tacc[:], texc_bc, op=ALU.max)
                        nc.vector.tensor_reduce(
                            tred[:, c * KC : (c + 1) * KC],
                            tval[:],
                            axis=AX.X,
                            op=ALU.min,
                        )
                        chunk_idx += 1

                    # phase 3: decode (none-accepted -> 0, clamp 1e5) and store
                    trm = pr.tile([PT, K], F32, tag="trm")
                    nc.vector.tensor_scalar_min(trm[:], tred[:], 1.0e5)
                    tfin = pr.tile([PT, K], F32, tag="tfin")
                    nc.vector.scalar_tensor_tensor(
                        tfin[:], tred[:], BIGF, trm[:], op0=ALU.is_lt, op1=ALU.mult
                    )
                    nc.sync.dma_start(res[sl, :], tfin[:])

    nc.compile()
    return nc


@functools.lru_cache(maxsize=4)
def _built(reps: int, gp_n: int):
    return _build(reps=reps, gp_n=gp_n)


def kernel(
    time_seqs,
    time_delta_seqs,
    type_seqs,
    e_unif,
    u,
    mu,
    alpha,
    beta,
    gamma,
    num_sample,
    _reps: int = 1,
    _gp_n: int = 5,
):
    e_unif = np.ascontiguousarray(np.asarray(e_unif, dtype=np.float32)).reshape(
        ROWS, E
    )
    u = np.ascontiguousarray(np.asarray(u, dtype=np.float32)).reshape(ROWS, K, E)
    tqf = np.ascontiguousarray(np.asarray(type_seqs).astype(np.float32)).reshape(ROWS)
    muf = np.ascontiguousarray(np.asarray(mu, dtype=np.float32))
    alf = np.ascontiguousarray(np.asarray(alpha, dtype=np.float32))
    bef = np.ascontiguousarray(np.asarray(beta, dtype=np.float32))
    gaf = np.ascontiguousarray(np.asarray(gamma, dtype=np.float32))
    arf = np.arange(NTYPES, dtype=np.float32)

    nc = _built(_reps, _gp_n)
    in_maps = []
    for c in range(NCORES):
        rs = slice(c * RPC, (c + 1) * RPC)
        in_maps.append(
            {
                "eu": e_unif[rs],
                "uu": u[rs],
                "tq": tqf[rs],
                "mu": muf,
                "al": alf,
                "be": bef,
                "ga": gaf,
                "ar": arf,
            }
        )
    out = run_bass_kernel_spmd(nc, in_maps, core_ids=list(range(NCORES)))
    res = np.concatenate([out.results[c]["res"] for c in range(NCORES)], axis=0)
    res = res.reshape(B, L, K)
    weights = np.full((B, L, K), 1.0 / float(num_sample), dtype=np.float32)
    return res, weights
